# revision 5
# baseline (speedup 1.0000x reference)
"""PSLoRA linear layer on 8 Trainium2 NeuronCores (Bass/Tile, bf16).

out[b] = x[b] @ W.T + bias + 0.5 * (x[b] @ lora_A[idx[b]]) @ lora_B.T

Sharding: data-parallel over batch (B=8 -> one batch element per core).
W / lora params are replicated; the per-core lora_A gather happens on host
(index has only 8 entries).

Per core, everything runs in bf16 (rel err ~3e-3 vs the 2e-2 gate): the
whole 16 MiB x^T stays resident in SBUF so W streams from HBM exactly
once (32 MiB bf16); the output is written back as bf16 (host casts to
f32). The LoRA delta and bias fold into the same PSUM accumulation group
as the base matmul via one extra K=33 matmul (32 axT rows + a ones row
paired with [0.5*B^T; bias]).

HW-measured constraints this version targets:
- matmuls accumulating bank-sequentially sustain ~79 ns (N=512 bf16) vs
  ~125 ns when 8 banks interleave per k-step -> inner loop is one full
  33-step accumulation per 128x512 tile, banks rotating per tile.
- each dma_start costs ~2 us fixed on its issuing engine's HWDGE ring
  (FIFO per engine) -> few, large, per-partition-contiguous transfers
  (56/body vs 180), with loads on the sync ring and stores on the
  scalar ring; output tiles stage into 1 MiB half-panel buffers.
"""
import sys
sys.path.insert(0, "/opt/trn_rl_repo")
import numpy as np

B, S, DIN, DOUT, R = 8, 2048, 4096, 4096, 32
LORA_SCALING = 16 / 32
KT = DIN // 128          # 32 contraction tiles
SB = S // 16 // 8        # 16 s-blocks
OB = DOUT // 512         # 8 output panels
XC = 8                   # x chunks (4 k-tiles each, 2 MiB)
XK = KT // XC
QPK = 8                  # k-tiles per W quarter-panel (1 MiB)
N_CORES = 8

_cache = {}


def _build(hw_loop=1):
    import concourse.bacc as bacc
    import concourse.mybir as mybir
    from concourse.tile import TileContext

    BF16 = mybir.dt.bfloat16
    F32 = mybir.dt.float32

    nc = bacc.Bacc()
    xT = nc.dram_tensor("xT", [128, KT * S], BF16, kind="ExternalInput")
    WT = nc.dram_tensor("WT", [128, OB * KT * 512], BF16, kind="ExternalInput")
    AbR = nc.dram_tensor("AbR", [128, KT * R], BF16, kind="ExternalInput")
    # rows 0-31: 0.5*lora_B.T, row 32: bias
    BTa = nc.dram_tensor("BTa", [R + 1, DOUT], BF16, kind="ExternalInput")
    ONES = nc.dram_tensor("ONES", [1, 512], BF16, kind="ExternalInput")
    # [OB, 128, SB, 512]: panel-major, partition-contiguous per panel half
    out = nc.dram_tensor("out", [OB * 128, 16 * 512], BF16,
                         kind="ExternalOutput")

    with TileContext(nc) as tc:
        with (
            tc.tile_pool(name="xp", bufs=XC) as xp,
            tc.tile_pool(name="wp", bufs=5) as wp,
            tc.tile_pool(name="cp", bufs=1) as cp,
            tc.tile_pool(name="axp", bufs=4) as axp,
            tc.tile_pool(name="op", bufs=2) as op_,
            tc.tile_pool(name="pp", bufs=1, space="PSUM") as pp,
        ):
            ab = cp.tile([128, KT * R], BF16, name="ab")
            nc.sync.dma_start(ab, AbR[:, :])
            bt = cp.tile([R + 1, DOUT], BF16, name="bt")
            nc.sync.dma_start(bt, BTa[:, :])
            ones = cp.tile([1, 512], BF16, name="ones")
            nc.sync.dma_start(ones, ONES[0:1, :])

            def xs(xt, k, lo, width):
                return xt[k // XK][:, (k % XK) * S + lo:(k % XK) * S + lo + width]

            def body():
                xt = []
                for j in range(XC):
                    t = xp.tile([128, XK * S], BF16, name="xq")
                    nc.sync.dma_start(t, xT[:, j * XK * S:(j + 1) * XK * S])
                    xt.append(t)
                # axT (transposed lora activations + ones row) per 512 cols
                axc = []
                for c in range(S // 512):
                    pa = pp.tile([R, 512], F32, name=f"ps{c}")
                    for k in range(KT):
                        nc.tensor.matmul(
                            pa, lhsT=ab[:, k * R:(k + 1) * R],
                            rhs=xs(xt, k, c * 512, 512),
                            start=(k == 0), stop=(k == KT - 1))
                    axt = axp.tile([R + 1, 512], BF16, name="axt")
                    nc.vector.tensor_copy(axt[0:R, :], pa)
                    nc.sync.dma_start(axt[R:R + 1, :], ONES[0:1, :])
                    axc.append(axt)
                # main panels: base matmul + fused lora delta + bias.
                # bank-sequential: one 128x512 tile accumulates 33 steps in
                # one PSUM bank; banks rotate per tile so evictions overlap.
                for ob in range(OB):
                    qp = []
                    for q in range(KT // QPK):
                        w = wp.tile([128, QPK * 512], BF16, name="wt")
                        off = (ob * KT + q * QPK) * 512
                        nc.sync.dma_start(w, WT[:, off:off + QPK * 512])
                        qp.append(w)
                    for half in range(2):
                        st = op_.tile([128, 8 * 512], BF16, name="st")
                        for i in range(8):
                            sbg = half * 8 + i
                            ps = pp.tile([128, 512], F32, name=f"ps{sbg % 8}")
                            col = sbg * 128
                            for k in range(KT):
                                nc.tensor.matmul(
                                    ps, lhsT=xs(xt, k, col, 128),
                                    rhs=qp[k // QPK][
                                        :, (k % QPK) * 512:(k % QPK + 1) * 512],
                                    start=(k == 0), stop=False)
                            nc.tensor.matmul(
                                ps,
                                lhsT=axc[sbg // 4][
                                    :, (sbg % 4) * 128:(sbg % 4 + 1) * 128],
                                rhs=bt[:, ob * 512:(ob + 1) * 512],
                                start=False, stop=True)
                            dst = st[:, i * 512:(i + 1) * 512]
                            if i % 2 == 0:
                                nc.vector.tensor_copy(dst, ps)
                            else:
                                nc.scalar.copy(dst, ps)
                        nc.scalar.dma_start(
                            out[ob * 128:(ob + 1) * 128,
                                half * 8 * 512:(half + 1) * 8 * 512], st)

            if hw_loop > 1:
                with tc.For_i(0, hw_loop, 1):
                    body()
            else:
                body()
    nc.finalize()
    return nc


def _prep_in_maps(input, weight, bias, lora_A, lora_B, labeler_index):
    import ml_dtypes
    bf16 = ml_dtypes.bfloat16

    x = np.asarray(input, dtype=np.float32)
    W = np.asarray(weight, dtype=np.float32)
    bias = np.asarray(bias, dtype=np.float32)
    lA = np.asarray(lora_A, dtype=np.float32)
    lB = np.asarray(lora_B, dtype=np.float32)
    idx = np.asarray(labeler_index).astype(np.int64)

    # W^T tiled as [128, OB, KT, 512] so a quarter-panel DMA is contiguous
    WTr = np.ascontiguousarray(
        W.T.reshape(KT, 128, OB, 512).transpose(1, 2, 0, 3)
    ).astype(bf16).reshape(128, OB * KT * 512)
    BTa = np.concatenate(
        [LORA_SCALING * lB.T, bias[None, :]], axis=0).astype(bf16)
    ones = np.ones((1, 512), dtype=bf16)

    in_maps = []
    for b in range(B):
        xTr = np.ascontiguousarray(
            x[b].T.reshape(KT, 128, S).transpose(1, 0, 2)
        ).astype(bf16).reshape(128, KT * S)
        AbR = np.ascontiguousarray(
            lA[idx[b]].reshape(KT, 128, R).transpose(1, 0, 2)
        ).astype(bf16).reshape(128, KT * R)
        in_maps.append({"xT": xTr, "WT": WTr, "AbR": AbR, "BTa": BTa,
                        "ONES": ones})
    return in_maps


def _assemble(raw):
    # raw: [OB*128, 16*512] bf16 -> [S, DOUT] f32
    arr = raw.reshape(OB, 128, 16, 512).transpose(2, 1, 0, 3)
    return np.ascontiguousarray(arr).reshape(S, DOUT).astype(np.float32)


def kernel(input, weight, bias, lora_A, lora_B, labeler_index):
    from concourse import bass_utils

    in_maps = _prep_in_maps(input, weight, bias, lora_A, lora_B, labeler_index)
    if "nc" not in _cache:
        _cache["nc"] = _build()
    last_err = None
    for attempt in range(3):
        try:
            res = bass_utils.run_bass_kernel_spmd(
                _cache["nc"], in_maps, core_ids=list(range(N_CORES)))
            return np.stack([_assemble(res.results[b]["out"])
                             for b in range(B)])
        except Exception as e:  # transient NRT wedge from a prior crashed run
            last_err = e
            if "UNRECOVERABLE" not in str(e) and "UNAVAILABLE" not in str(e):
                raise
    raise last_err


# revision 7
# speedup vs baseline: 1.0290x; 1.0290x over previous
"""PSLoRA linear layer on 8 Trainium2 NeuronCores (Bass/Tile, bf16).

out[b] = x[b] @ W.T + bias + 0.5 * (x[b] @ lora_A[idx[b]]) @ lora_B.T

Sharding: data-parallel over batch (B=8 -> one batch element per core).
W / lora params are replicated; the per-core lora_A gather happens on host
(index has only 8 entries).

Per core, everything runs in bf16 (rel err ~3e-3 vs the 2e-2 gate): the
whole 16 MiB x^T stays resident in SBUF so W streams from HBM exactly
once; the output is computed TRANSPOSED ([DOUT,S] tiles) and written
back as bf16 (host transposes and casts to f32).

The transposed form makes the W-block the stationary (weights) operand,
shared by the 4 matmuls that cover S at each k step: HW-measured, a
stream of N=512 bf16 matmuls runs ~188 ns each when the stationary
operand repeats across 4 consecutive matmuls vs ~270 ns when it changes
every matmul. o-blocks alternate between PSUM bank sets {0-3}/{4-7} so
evictions overlap the next block's accumulation. The LoRA delta and
bias fold into the same accumulation group via one extra K=33 matmul
(32 axT rows + a ones row paired with [0.5*B^T; bias], also stationary
per o-block). DMA rings are split by traffic class (x chunks: gpsimd,
W: sync, output + ones rows: scalar).
"""
import sys
sys.path.insert(0, "/opt/trn_rl_repo")
import numpy as np

B, S, DIN, DOUT, R = 8, 2048, 4096, 4096, 32
LORA_SCALING = 16 / 32
KT = DIN // 128          # 32 contraction tiles
OB2 = DOUT // 128        # 32 output o-blocks
XC = 8                   # x chunks (4 k-tiles each, 2 MiB)
XK = KT // XC
N_CORES = 8

_cache = {}


def _build(hw_loop=1):
    import concourse.bacc as bacc
    import concourse.mybir as mybir
    from concourse.tile import TileContext

    BF16 = mybir.dt.bfloat16
    F32 = mybir.dt.float32

    nc = bacc.Bacc()
    xT = nc.dram_tensor("xT", [128, KT * S], BF16, kind="ExternalInput")
    # [p, ob, k, m]: W[ob*128+m, k*128+p]
    WTo = nc.dram_tensor("WTo", [128, OB2 * KT * 128], BF16,
                         kind="ExternalInput")
    AbR = nc.dram_tensor("AbR", [128, KT * R], BF16, kind="ExternalInput")
    # rows 0-31: 0.5*lora_B.T, row 32: bias
    BTa = nc.dram_tensor("BTa", [R + 1, DOUT], BF16, kind="ExternalInput")
    ONES = nc.dram_tensor("ONES", [1, 512], BF16, kind="ExternalInput")
    # [p, ob, s]: outT[ob*128+p, s]
    out = nc.dram_tensor("out", [128, OB2, S], BF16, kind="ExternalOutput")

    with TileContext(nc) as tc:
        with (
            tc.tile_pool(name="xp", bufs=XC) as xp,
            tc.tile_pool(name="wp", bufs=3) as wp,
            tc.tile_pool(name="cp", bufs=1) as cp,
            tc.tile_pool(name="axp", bufs=4) as axp,
            tc.tile_pool(name="op", bufs=2) as op_,
            tc.tile_pool(name="pp", bufs=1, space="PSUM") as pp,
        ):
            ab = cp.tile([128, KT * R], BF16, name="ab")
            nc.sync.dma_start(ab, AbR[:, :])
            bt = cp.tile([R + 1, DOUT], BF16, name="bt")
            nc.sync.dma_start(bt, BTa[:, :])

            def xs(xt, k, lo, width):
                return xt[k // XK][:, (k % XK) * S + lo:(k % XK) * S + lo + width]

            def body():
                xt = []
                for j in range(XC):
                    t = xp.tile([128, XK * S], BF16, name="xq")
                    nc.gpsimd.dma_start(
                        t, xT[:, j * XK * S:(j + 1) * XK * S])
                    xt.append(t)
                # axT (transposed lora activations + ones row) per 512 cols
                axc = []
                for c in range(S // 512):
                    pa = pp.tile([R, 512], F32, name=f"ps{c}")
                    for k in range(KT):
                        nc.tensor.matmul(
                            pa, lhsT=ab[:, k * R:(k + 1) * R],
                            rhs=xs(xt, k, c * 512, 512),
                            start=(k == 0), stop=(k == KT - 1))
                    axt = axp.tile([R + 1, 512], BF16, name="axt")
                    nc.vector.tensor_copy(axt[0:R, :], pa)
                    nc.scalar.dma_start(axt[R:R + 1, :], ONES[0:1, :])
                    axc.append(axt)
                # main: per o-block pair, k-outer with the W-block stationary
                # across the 4 S-chunks; banks {0-3}/{4-7} alternate per ob.
                for j in range(OB2 // 2):
                    wb = wp.tile([128, 2 * KT * 128], BF16, name="wt")
                    off = (2 * j) * KT * 128
                    nc.sync.dma_start(wb, WTo[:, off:off + 2 * KT * 128])
                    for par in range(2):
                        ob = 2 * j + par
                        ps = [pp.tile([128, 512], F32, name=f"ps{par * 4 + c}")
                              for c in range(4)]
                        for k in range(KT):
                            wsl = wb[:, (par * KT + k) * 128:
                                     (par * KT + k + 1) * 128]
                            for c in range(4):
                                nc.tensor.matmul(
                                    ps[c], lhsT=wsl,
                                    rhs=xs(xt, k, c * 512, 512),
                                    start=(k == 0), stop=False)
                        btsl = bt[:, ob * 128:(ob + 1) * 128]
                        for c in range(4):
                            nc.tensor.matmul(
                                ps[c], lhsT=btsl, rhs=axc[c][:, :],
                                start=False, stop=True)
                        if par == 0:
                            st = op_.tile([128, 2, 4 * 512], BF16, name="st")
                        for c in range(4):
                            dst = st[:, par, c * 512:(c + 1) * 512]
                            if c % 2 == 0:
                                nc.vector.tensor_copy(dst, ps[c])
                            else:
                                nc.scalar.copy(dst, ps[c])
                        if par == 1:
                            nc.scalar.dma_start(
                                out[:, 2 * j:2 * j + 2, :], st[:, :, :])

            if hw_loop > 1:
                with tc.For_i(0, hw_loop, 1):
                    body()
            else:
                body()
    nc.finalize()
    return nc


def _prep_in_maps(input, weight, bias, lora_A, lora_B, labeler_index):
    import ml_dtypes
    bf16 = ml_dtypes.bfloat16

    x = np.asarray(input, dtype=np.float32)
    W = np.asarray(weight, dtype=np.float32)
    bias = np.asarray(bias, dtype=np.float32)
    lA = np.asarray(lora_A, dtype=np.float32)
    lB = np.asarray(lora_B, dtype=np.float32)
    idx = np.asarray(labeler_index).astype(np.int64)

    # [p, ob, k, m] = W[ob*128+m, k*128+p]
    WTo = np.ascontiguousarray(
        W.reshape(OB2, 128, KT, 128).transpose(3, 0, 2, 1)
    ).astype(bf16).reshape(128, OB2 * KT * 128)
    BTa = np.concatenate(
        [LORA_SCALING * lB.T, bias[None, :]], axis=0).astype(bf16)
    ones = np.ones((1, 512), dtype=bf16)

    in_maps = []
    for b in range(B):
        xTr = np.ascontiguousarray(
            x[b].T.reshape(KT, 128, S).transpose(1, 0, 2)
        ).astype(bf16).reshape(128, KT * S)
        AbR = np.ascontiguousarray(
            lA[idx[b]].reshape(KT, 128, R).transpose(1, 0, 2)
        ).astype(bf16).reshape(128, KT * R)
        in_maps.append({"xT": xTr, "WTo": WTo, "AbR": AbR, "BTa": BTa,
                        "ONES": ones})
    return in_maps


def _assemble(raw):
    # raw: [128, OB2, S] bf16, [p, ob, s] = out[s, ob*128+p] -> [S, DOUT] f32
    return np.ascontiguousarray(raw.transpose(2, 1, 0)).reshape(
        S, DOUT).astype(np.float32)


def kernel(input, weight, bias, lora_A, lora_B, labeler_index):
    from concourse import bass_utils

    in_maps = _prep_in_maps(input, weight, bias, lora_A, lora_B, labeler_index)
    if "nc" not in _cache:
        _cache["nc"] = _build()
    last_err = None
    for attempt in range(3):
        try:
            res = bass_utils.run_bass_kernel_spmd(
                _cache["nc"], in_maps, core_ids=list(range(N_CORES)))
            return np.stack([_assemble(res.results[b]["out"])
                             for b in range(B)])
        except Exception as e:  # transient NRT wedge from a prior crashed run
            last_err = e
            if "UNRECOVERABLE" not in str(e) and "UNAVAILABLE" not in str(e):
                raise
    raise last_err


# revision 14
# speedup vs baseline: 1.0487x; 1.0192x over previous
"""PSLoRA linear layer on 8 Trainium2 NeuronCores (Bass/Tile, bf16).

out[b] = x[b] @ W.T + bias + 0.5 * (x[b] @ lora_A[idx[b]]) @ lora_B.T

Sharding: data-parallel over batch (B=8 -> one batch element per core).
W / lora params are replicated; the per-core lora_A gather happens on host
(index has only 8 entries).

Per core, everything runs in bf16 (rel err ~3e-3 vs the 2e-2 gate): the
whole 16 MiB x^T stays resident in SBUF so W streams from HBM exactly
once; the output is computed TRANSPOSED ([DOUT,S] tiles) and written
back as bf16 (host transposes and casts to f32).

The transposed form makes the W-block the stationary (weights) operand,
shared by the 4 matmuls that cover S at each k step: HW-measured, a
stream of N=512 bf16 matmuls runs ~188 ns each when the stationary
operand repeats across 4 consecutive matmuls vs ~270 ns when it changes
every matmul. o-blocks alternate between PSUM bank sets {0-3}/{4-7} so
evictions overlap the next block's accumulation. The LoRA delta and
bias fold into the same accumulation group via one extra K=33 matmul
(32 axT rows + a ones row paired with [0.5*B^T; bias], also stationary
per o-block). DMA rings are split by traffic class (x chunks: gpsimd,
W: sync, output + ones rows: scalar).
"""
import sys
sys.path.insert(0, "/opt/trn_rl_repo")
import numpy as np

B, S, DIN, DOUT, R = 8, 2048, 4096, 4096, 32
LORA_SCALING = 16 / 32
KT = DIN // 128          # 32 contraction tiles
OB2 = DOUT // 128        # 32 output o-blocks
NF8 = 8                  # leading k-tiles computed in fp8 DoubleRow
NP8 = NF8 // 2           # fp8 k-pairs
KB = KT - NF8            # bf16 k-tiles
XC = 6                   # bf16 x chunks (4 k-tiles each)
XK = KB // XC
WSCALE = 64.0            # W-side operands scaled x64 (fp8 normal range)
N_CORES = 8

_cache = {}


def _build(hw_loop=1):
    import concourse.bacc as bacc
    import concourse.mybir as mybir
    from concourse.tile import TileContext

    BF16 = mybir.dt.bfloat16
    FP8 = mybir.dt.float8e4
    F32 = mybir.dt.float32

    nc = bacc.Bacc()
    # bf16 x^T, k-tiles NF8..KT-1: [p, kb, s]
    xT = nc.dram_tensor("xT", [128, KB * S], BF16, kind="ExternalInput")
    # fp8 x^T hi/lo halves, k-tiles 0..NF8-1 as DR pairs: [p, t, a, s]
    x8h = nc.dram_tensor("x8h", [128, NP8 * 2 * S], FP8, kind="ExternalInput")
    x8l = nc.dram_tensor("x8l", [128, NP8 * 2 * S], FP8, kind="ExternalInput")
    # [p, ob, kb, m]: 64*W[ob*128+m, (NF8+kb)*128+p]
    WTo = nc.dram_tensor("WTo", [128, OB2 * KB * 128], BF16,
                         kind="ExternalInput")
    # [p, ob, t, a, m]: fp8(64*W[ob*128+m, (2t+a)*128+p])
    W8o = nc.dram_tensor("W8o", [128, OB2 * NP8 * 2 * 128], FP8,
                         kind="ExternalInput")
    # [p, kb, r]: 64*A (bf16 part) / [p, t, a, r]: fp8(64*A) (fp8 part)
    AbR = nc.dram_tensor("AbR", [128, KB * R], BF16, kind="ExternalInput")
    A8R = nc.dram_tensor("A8R", [128, NP8 * 2 * R], FP8, kind="ExternalInput")
    # rows 0-31: 64*0.5*lora_B.T, row 32: 64*bias
    BTa = nc.dram_tensor("BTa", [R + 1, DOUT], BF16, kind="ExternalInput")
    ONES = nc.dram_tensor("ONES", [1, 512], BF16, kind="ExternalInput")
    # [p, ob, s]: outT[ob*128+p, s]
    out = nc.dram_tensor("out", [128, OB2, S], BF16, kind="ExternalOutput")

    with TileContext(nc) as tc:
        with (
            tc.tile_pool(name="xp", bufs=XC) as xp,
            tc.tile_pool(name="x8p", bufs=2) as x8p,
            tc.tile_pool(name="wp", bufs=3) as wp,
            tc.tile_pool(name="cp", bufs=1) as cp,
            tc.tile_pool(name="axp", bufs=4) as axp,
            tc.tile_pool(name="op", bufs=2) as op_,
            tc.tile_pool(name="pp", bufs=1, space="PSUM") as pp,
        ):
            ab = cp.tile([128, KB * R], BF16, name="ab")
            nc.sync.dma_start(ab, AbR[:, :])
            a8 = cp.tile([128, NP8, 2, R], FP8, name="a8")
            nc.sync.dma_start(
                a8, A8R[:, :].rearrange("p (t a r) -> p t a r", t=NP8, a=2))
            bt = cp.tile([R + 1, DOUT], BF16, name="bt")
            nc.sync.dma_start(bt, BTa[:, :])

            def xs(xt, kb, lo, width):
                return xt[kb // XK][:, (kb % XK) * S + lo:
                                    (kb % XK) * S + lo + width]

            def body():
                xt = []
                for j in range(XC):
                    t = xp.tile([128, XK * S], BF16, name="xq")
                    nc.gpsimd.dma_start(
                        t, xT[:, j * XK * S:(j + 1) * XK * S])
                    xt.append(t)
                x8 = []
                for src in (x8h, x8l):
                    t = x8p.tile([128, NP8, 2, S], FP8, name="x8")
                    nc.gpsimd.dma_start(
                        t, src[:, :].rearrange("p (t a s) -> p t a s",
                                               t=NP8, a=2))
                    x8.append(t)
                # axT (transposed lora activations + ones row) per 512 cols;
                # psum carries 64*ax, eviction scales by 1/64
                axc = []
                for c in range(S // 512):
                    pa = pp.tile([R, 512], F32, name=f"ps{c}")
                    for t in range(NP8):
                        nc.tensor.matmul(
                            pa, lhsT=a8[:, t, :, :],
                            rhs=x8[0][:, t, :, c * 512:(c + 1) * 512],
                            start=(t == 0), stop=False,
                            perf_mode=mybir.MatmulPerfMode.DoubleRow)
                    for kb in range(KB):
                        nc.tensor.matmul(
                            pa, lhsT=ab[:, kb * R:(kb + 1) * R],
                            rhs=xs(xt, kb, c * 512, 512),
                            start=False, stop=(kb == KB - 1))
                    axt = axp.tile([R + 1, 512], BF16, name="axt")
                    nc.scalar.mul(axt[0:R, :], pa, 1.0 / WSCALE)
                    nc.scalar.dma_start(axt[R:R + 1, :], ONES[0:1, :])
                    axc.append(axt)
                # main: per o-block pair, k-outer with the W-block stationary
                # across the 4 S-chunks; banks {0-3}/{4-7} alternate per ob.
                # fp8 DR pairs first, then bf16 k-tiles, then lora+bias;
                # everything in psum is 64x, evictions scale by 1/64.
                for j in range(OB2 // 2):
                    wb = wp.tile([128, 2 * KB * 128], BF16, name="wt")
                    off = (2 * j) * KB * 128
                    nc.sync.dma_start(wb, WTo[:, off:off + 2 * KB * 128])
                    w8 = wp.tile([128, 2, NP8, 2, 128], FP8, name="w8")
                    nc.sync.dma_start(
                        w8, W8o[:, 2 * j * NP8 * 2 * 128:
                                (2 * j + 2) * NP8 * 2 * 128].rearrange(
                            "p (o t a m) -> p o t a m", o=2, t=NP8, a=2))
                    for par in range(2):
                        ob = 2 * j + par
                        ps = [pp.tile([128, 512], F32, name=f"ps{par * 4 + c}")
                              for c in range(4)]
                        for t in range(NP8):
                            w8sl = w8[:, par, t, :, :]
                            for half in range(2):
                                for c in range(4):
                                    nc.tensor.matmul(
                                        ps[c], lhsT=w8sl,
                                        rhs=x8[half][:, t, :,
                                                     c * 512:(c + 1) * 512],
                                        start=(t == 0 and half == 0),
                                        stop=False,
                                        perf_mode=mybir.MatmulPerfMode.DoubleRow)
                        for kb in range(KB):
                            wsl = wb[:, (par * KB + kb) * 128:
                                     (par * KB + kb + 1) * 128]
                            for c in range(4):
                                nc.tensor.matmul(
                                    ps[c], lhsT=wsl,
                                    rhs=xs(xt, kb, c * 512, 512),
                                    start=False, stop=False)
                        btsl = bt[:, ob * 128:(ob + 1) * 128]
                        for c in range(4):
                            nc.tensor.matmul(
                                ps[c], lhsT=btsl, rhs=axc[c][:, :],
                                start=False, stop=True)
                        if par == 0:
                            st = op_.tile([128, 2, 4 * 512], BF16, name="st")
                        for c in range(4):
                            dst = st[:, par, c * 512:(c + 1) * 512]
                            if c % 2 == 0:
                                nc.vector.tensor_scalar_mul(
                                    dst, ps[c], 1.0 / WSCALE)
                            else:
                                nc.scalar.mul(dst, ps[c], 1.0 / WSCALE)
                        if par == 1:
                            nc.scalar.dma_start(
                                out[:, 2 * j:2 * j + 2, :], st[:, :, :])

            if hw_loop > 1:
                with tc.For_i(0, hw_loop, 1):
                    body()
            else:
                body()
    nc.finalize()
    return nc


def _prep_in_maps(input, weight, bias, lora_A, lora_B, labeler_index):
    import ml_dtypes
    bf16 = ml_dtypes.bfloat16
    fp8 = ml_dtypes.float8_e4m3fn

    x = np.asarray(input, dtype=np.float32)
    W = np.asarray(weight, dtype=np.float32)
    bias = np.asarray(bias, dtype=np.float32)
    lA = np.asarray(lora_A, dtype=np.float32)
    lB = np.asarray(lora_B, dtype=np.float32)
    idx = np.asarray(labeler_index).astype(np.int64)
    ksp = NF8 * 128      # DIN split point: fp8 below, bf16 above

    # [p, ob, kb, m] = 64*W[ob*128+m, ksp + kb*128+p]
    WTo = np.ascontiguousarray(
        (WSCALE * W[:, ksp:]).reshape(OB2, 128, KB, 128).transpose(3, 0, 2, 1)
    ).astype(bf16).reshape(128, OB2 * KB * 128)
    # [p, ob, t, a, m] = fp8(64*W[ob*128+m, (2t+a)*128+p])
    W8o = np.ascontiguousarray(
        (WSCALE * W[:, :ksp]).reshape(OB2, 128, NP8, 2, 128)
        .transpose(4, 0, 2, 3, 1)
    ).astype(fp8).reshape(128, OB2 * NP8 * 2 * 128)
    BTa = np.concatenate(
        [WSCALE * LORA_SCALING * lB.T, WSCALE * bias[None, :]],
        axis=0).astype(bf16)
    ones = np.ones((1, 512), dtype=bf16)

    in_maps = []
    for b in range(B):
        xb = x[b]                       # [S, DIN]
        xhi = xb[:, :ksp].astype(fp8)
        xlo = (xb[:, :ksp] - xhi.astype(np.float32)).astype(fp8)
        # [p, t, a, s] = x[s, (2t+a)*128+p]
        x8h = np.ascontiguousarray(
            xhi.reshape(S, NP8, 2, 128).transpose(3, 1, 2, 0)
        ).reshape(128, NP8 * 2 * S)
        x8l = np.ascontiguousarray(
            xlo.reshape(S, NP8, 2, 128).transpose(3, 1, 2, 0)
        ).reshape(128, NP8 * 2 * S)
        xTr = np.ascontiguousarray(
            xb[:, ksp:].T.reshape(KB, 128, S).transpose(1, 0, 2)
        ).astype(bf16).reshape(128, KB * S)
        Ab = WSCALE * lA[idx[b]]        # [DIN, R]
        AbR = np.ascontiguousarray(
            Ab[ksp:].reshape(KB, 128, R).transpose(1, 0, 2)
        ).astype(bf16).reshape(128, KB * R)
        A8R = np.ascontiguousarray(
            Ab[:ksp].reshape(NP8, 2, 128, R).transpose(2, 0, 1, 3)
        ).astype(fp8).reshape(128, NP8 * 2 * R)
        in_maps.append({"xT": xTr, "x8h": x8h, "x8l": x8l, "WTo": WTo,
                        "W8o": W8o, "AbR": AbR, "A8R": A8R, "BTa": BTa,
                        "ONES": ones})
    return in_maps


def _assemble(raw):
    # raw: [128, OB2, S] bf16, [p, ob, s] = out[s, ob*128+p] -> [S, DOUT] f32
    return np.ascontiguousarray(raw.transpose(2, 1, 0)).reshape(
        S, DOUT).astype(np.float32)


def kernel(input, weight, bias, lora_A, lora_B, labeler_index):
    from concourse import bass_utils

    in_maps = _prep_in_maps(input, weight, bias, lora_A, lora_B, labeler_index)
    if "nc" not in _cache:
        _cache["nc"] = _build()
    last_err = None
    for attempt in range(3):
        try:
            res = bass_utils.run_bass_kernel_spmd(
                _cache["nc"], in_maps, core_ids=list(range(N_CORES)))
            return np.stack([_assemble(res.results[b]["out"])
                             for b in range(B)])
        except Exception as e:  # transient NRT wedge from a prior crashed run
            last_err = e
            if "UNRECOVERABLE" not in str(e) and "UNAVAILABLE" not in str(e):
                raise
    raise last_err


# revision 15
# speedup vs baseline: 1.0785x; 1.0284x over previous
"""PSLoRA linear layer on 8 Trainium2 NeuronCores (Bass/Tile, bf16).

out[b] = x[b] @ W.T + bias + 0.5 * (x[b] @ lora_A[idx[b]]) @ lora_B.T

Sharding: data-parallel over batch (B=8 -> one batch element per core).
W / lora params are replicated; the per-core lora_A gather happens on host
(index has only 8 entries).

Per core, everything runs in bf16 (rel err ~3e-3 vs the 2e-2 gate): the
whole 16 MiB x^T stays resident in SBUF so W streams from HBM exactly
once (32 MiB bf16); the output is written back as bf16 (host casts to
f32). The LoRA delta and bias fold into the same PSUM accumulation group
as the base matmul via one extra K=33 matmul (32 axT rows + a ones row
paired with [0.5*B^T; bias]).

Engine/queue layout (HWDGE rings are FIFO per issuing engine, so each
traffic class gets its own engine): x chunks on gpsimd, W quarter-panels
on sync, output stores + the per-chunk ones row on scalar. The final
panel runs k-outer (8-bank interleave) so the x chunks free early enough
for the next iteration's reloads to overlap its tail; all other panels
run bank-sequential (one full 33-step accumulation per 128x512 tile,
banks rotating per tile so evictions overlap the next 7 tiles).
"""
import sys
sys.path.insert(0, "/opt/trn_rl_repo")
import numpy as np

B, S, DIN, DOUT, R = 8, 2048, 4096, 4096, 32
LORA_SCALING = 16 / 32
KT = DIN // 128          # 32 contraction tiles
OB = DOUT // 512         # 8 output panels
XC = 8                   # x chunks (4 k-tiles each, 2 MiB)
XK = KT // XC
QPK = 8                  # k-tiles per W quarter-panel (1 MiB)
N_CORES = 8

_cache = {}


def _build(hw_loop=1, skip_in_dma=False, skip_compute=False):
    import concourse.bacc as bacc
    import concourse.mybir as mybir
    from concourse.tile import TileContext

    BF16 = mybir.dt.bfloat16
    F32 = mybir.dt.float32

    nc = bacc.Bacc()
    xT = nc.dram_tensor("xT", [128, KT * S], BF16, kind="ExternalInput")
    WT = nc.dram_tensor("WT", [128, OB * KT * 512], BF16, kind="ExternalInput")
    AbR = nc.dram_tensor("AbR", [128, KT * R], BF16, kind="ExternalInput")
    # rows 0-31: 0.5*lora_B.T, row 32: bias
    BTa = nc.dram_tensor("BTa", [R + 1, DOUT], BF16, kind="ExternalInput")
    ONES = nc.dram_tensor("ONES", [1, 512], BF16, kind="ExternalInput")
    # [OB, 128, 16, 512]: panel-major, partition-contiguous per panel half
    out = nc.dram_tensor("out", [OB * 128, 16 * 512], BF16,
                         kind="ExternalOutput")

    with TileContext(nc) as tc:
        with (
            tc.tile_pool(name="xp", bufs=XC) as xp,
            tc.tile_pool(name="wp", bufs=5) as wp,
            tc.tile_pool(name="cp", bufs=1) as cp,
            tc.tile_pool(name="axp", bufs=4) as axp,
            tc.tile_pool(name="op", bufs=2) as op_,
            tc.tile_pool(name="pp", bufs=1, space="PSUM") as pp,
        ):
            ab = cp.tile([128, KT * R], BF16, name="ab")
            nc.sync.dma_start(ab, AbR[:, :])
            bt = cp.tile([R + 1, DOUT], BF16, name="bt")
            nc.sync.dma_start(bt, BTa[:, :])

            xt_pre = []
            if skip_in_dma:  # timing variant: x/W resident, loaded once
                for j in range(XC):
                    t = xp.tile([128, XK * S], BF16, name="xq")
                    nc.sync.dma_start(t, xT[:, j * XK * S:(j + 1) * XK * S])
                    xt_pre.append(t)
                wq_pre = []
                for q in range(KT // QPK):
                    w = wp.tile([128, QPK * 512], BF16, name="wt")
                    nc.sync.dma_start(w, WT[:, (q * QPK) * 512:
                                            (q * QPK + QPK) * 512])
                    wq_pre.append(w)

            def xs(xt, k, lo, width):
                return xt[k // XK][:, (k % XK) * S + lo:(k % XK) * S + lo + width]

            def body():
                if skip_in_dma:
                    xt = xt_pre
                else:
                    xt = []
                    for j in range(XC):
                        t = xp.tile([128, XK * S], BF16, name="xq")
                        nc.gpsimd.dma_start(
                            t, xT[:, j * XK * S:(j + 1) * XK * S])
                        xt.append(t)
                if skip_compute:
                    if not skip_in_dma:
                        for ob in range(OB):
                            for q in range(KT // QPK):
                                w = wp.tile([128, QPK * 512], BF16, name="wt")
                                off = (ob * KT + q * QPK) * 512
                                nc.sync.dma_start(
                                    w, WT[:, off:off + QPK * 512])
                    return
                # axT (transposed lora activations + ones row) per 512 cols
                axc = []
                for c in range(S // 512):
                    pa = pp.tile([R, 512], F32, name=f"ps{c}")
                    for k in range(KT):
                        nc.tensor.matmul(
                            pa, lhsT=ab[:, k * R:(k + 1) * R],
                            rhs=xs(xt, k, c * 512, 512),
                            start=(k == 0), stop=(k == KT - 1))
                    axt = axp.tile([R + 1, 512], BF16, name="axt")
                    nc.vector.tensor_copy(axt[0:R, :], pa)
                    nc.scalar.dma_start(axt[R:R + 1, :], ONES[0:1, :])
                    axc.append(axt)

                def lora_mm(ps, sbg, ob):
                    nc.tensor.matmul(
                        ps,
                        lhsT=axc[sbg // 4][
                            :, (sbg % 4) * 128:(sbg % 4 + 1) * 128],
                        rhs=bt[:, ob * 512:(ob + 1) * 512],
                        start=False, stop=True)

                def evict(st, i, ps):
                    dst = st[:, i * 512:(i + 1) * 512]
                    if i % 2 == 0:
                        nc.vector.tensor_copy(dst, ps)
                    else:
                        nc.scalar.copy(dst, ps)

                # main panels: base matmul + fused lora delta + bias
                for ob in range(OB):
                    qp = []
                    for q in range(KT // QPK):
                        w = wp.tile([128, QPK * 512], BF16, name="wt")
                        off = (ob * KT + q * QPK) * 512
                        nc.sync.dma_start(w, WT[:, off:off + QPK * 512])
                        qp.append(w)

                    def wsrc(k):
                        return qp[k // QPK][:, (k % QPK) * 512:(k % QPK + 1) * 512]

                    if ob < OB - 1:
                        # bank-sequential: full accumulation per tile
                        for half in range(2):
                            st = op_.tile([128, 8 * 512], BF16, name="st")
                            for i in range(8):
                                sbg = half * 8 + i
                                ps = pp.tile([128, 512], F32,
                                             name=f"ps{sbg % 8}")
                                col = sbg * 128
                                for k in range(KT):
                                    nc.tensor.matmul(
                                        ps, lhsT=xs(xt, k, col, 128),
                                        rhs=wsrc(k),
                                        start=(k == 0), stop=False)
                                lora_mm(ps, sbg, ob)
                                evict(st, i, ps)
                            nc.scalar.dma_start(
                                out[ob * 128:(ob + 1) * 128,
                                    half * 8 * 512:(half + 1) * 8 * 512], st)
                    else:
                        # last panel k-outer so x chunks free early for the
                        # next iteration's reloads
                        for half in range(2):
                            st = op_.tile([128, 8 * 512], BF16, name="st")
                            ps = [pp.tile([128, 512], F32, name=f"ps{i}")
                                  for i in range(8)]
                            for k in range(KT):
                                for i in range(8):
                                    col = (half * 8 + i) * 128
                                    nc.tensor.matmul(
                                        ps[i], lhsT=xs(xt, k, col, 128),
                                        rhs=wsrc(k),
                                        start=(k == 0), stop=False)
                            for i in range(8):
                                lora_mm(ps[i], half * 8 + i, ob)
                                evict(st, i, ps[i])
                            nc.scalar.dma_start(
                                out[ob * 128:(ob + 1) * 128,
                                    half * 8 * 512:(half + 1) * 8 * 512], st)

            if hw_loop > 1:
                with tc.For_i(0, hw_loop, 1):
                    body()
            else:
                body()
    nc.finalize()
    return nc


def _prep_in_maps(input, weight, bias, lora_A, lora_B, labeler_index):
    import ml_dtypes
    bf16 = ml_dtypes.bfloat16

    x = np.asarray(input, dtype=np.float32)
    W = np.asarray(weight, dtype=np.float32)
    bias = np.asarray(bias, dtype=np.float32)
    lA = np.asarray(lora_A, dtype=np.float32)
    lB = np.asarray(lora_B, dtype=np.float32)
    idx = np.asarray(labeler_index).astype(np.int64)

    # W^T tiled as [128, OB, KT, 512] so a quarter-panel DMA is contiguous
    WTr = np.ascontiguousarray(
        W.T.reshape(KT, 128, OB, 512).transpose(1, 2, 0, 3)
    ).astype(bf16).reshape(128, OB * KT * 512)
    BTa = np.concatenate(
        [LORA_SCALING * lB.T, bias[None, :]], axis=0).astype(bf16)
    ones = np.ones((1, 512), dtype=bf16)

    in_maps = []
    for b in range(B):
        xTr = np.ascontiguousarray(
            x[b].T.reshape(KT, 128, S).transpose(1, 0, 2)
        ).astype(bf16).reshape(128, KT * S)
        AbR = np.ascontiguousarray(
            lA[idx[b]].reshape(KT, 128, R).transpose(1, 0, 2)
        ).astype(bf16).reshape(128, KT * R)
        in_maps.append({"xT": xTr, "WT": WTr, "AbR": AbR, "BTa": BTa,
                        "ONES": ones})
    return in_maps


def _assemble(raw):
    # raw: [OB*128, 16*512] bf16 -> [S, DOUT] f32
    arr = raw.reshape(OB, 128, 16, 512).transpose(2, 1, 0, 3)
    return np.ascontiguousarray(arr).reshape(S, DOUT).astype(np.float32)


def kernel(input, weight, bias, lora_A, lora_B, labeler_index):
    from concourse import bass_utils

    in_maps = _prep_in_maps(input, weight, bias, lora_A, lora_B, labeler_index)
    if "nc" not in _cache:
        _cache["nc"] = _build()
    last_err = None
    for attempt in range(3):
        try:
            res = bass_utils.run_bass_kernel_spmd(
                _cache["nc"], in_maps, core_ids=list(range(N_CORES)))
            return np.stack([_assemble(res.results[b]["out"])
                             for b in range(B)])
        except Exception as e:  # transient NRT wedge from a prior crashed run
            last_err = e
            if "UNRECOVERABLE" not in str(e) and "UNAVAILABLE" not in str(e):
                raise
    raise last_err
